# revision 11
# baseline (speedup 1.0000x reference)
"""Multi-head self-attention Trainium2 kernel, sharded over 8 NeuronCores.

Sharding: core = (batch, head_group): 2 batches x 4 head-groups (4 heads each).
Each core computes qkv for its batch restricted to its heads (tensor-parallel
column slice), full-sequence attention for those heads, and a row-parallel
slice of the output projection, producing a partial [T, C] output.
Host: out[b] = sum of the 4 head-group partials + b_proj.
"""

import math
import sys

import numpy as np

sys.path.insert(0, "/opt/trn_rl_repo")

import concourse.bacc as bacc
import concourse.bass as bass
import concourse.tile as tile
from concourse import mybir
from concourse.bass_utils import run_bass_kernel_spmd

B, T, C = 2, 2048, 1024
NH, DH = 16, 64
HG = 4                 # heads per core
DL = HG * DH           # 256 local qk channels
DV = HG * (DH + 1)     # 260: v columns + one ones-column per head
N_CORES = 8

F32 = mybir.dt.float32
F32R = mybir.dt.float32r
F16 = mybir.dt.float16

SCALE = 1.0 / math.sqrt(DH)


def build_bass():
    nc = bacc.Bacc("TRN2", target_bir_lowering=False, debug=False)

    x_in = nc.declare_dram_parameter("x_loc", [T, C], F32, isOutput=False)
    w_qk = nc.declare_dram_parameter("w_qk", [C, 2 * DL], F32R, isOutput=False)
    w_v = nc.declare_dram_parameter("w_v_ext", [C, DV], F32R, isOutput=False)
    b_qk = nc.declare_dram_parameter("b_qk", [1, 2 * DL], F32R, isOutput=False)
    b_v = nc.declare_dram_parameter("b_v_ext", [1, DV], F32R, isOutput=False)
    w_p = nc.declare_dram_parameter("w_proj_loc", [DL, C], F16, isOutput=False)
    iden = nc.declare_dram_parameter("identity", [128, 128], F32, isOutput=False)
    ones = nc.declare_dram_parameter("ones_row", [1, T], F32R, isOutput=False)
    out = nc.declare_dram_parameter("out_partial", [T, C], F32, isOutput=True)

    Exp = mybir.ActivationFunctionType.Exp

    with tile.TileContext(nc) as tc:
        with (
            tc.tile_pool(name="singles", bufs=1) as singles,
            tc.tile_pool(name="xload", bufs=5) as xload,
            tc.tile_pool(name="pt", bufs=3) as ptp,
            tc.tile_pool(name="osmall", bufs=6) as osmall,
            tc.tile_pool(name="psmm", bufs=4, space="PSUM") as psmm,
            tc.tile_pool(name="psav", bufs=4, space="PSUM") as psav,
        ):
            iden_sb = singles.tile([128, 128], F32)
            nc.sync.dma_start(out=iden_sb[:], in_=iden[:])
            iden16 = singles.tile([128, 128], F16)
            nc.vector.tensor_copy(iden16[:], iden_sb[:])
            ones_sb = singles.tile([1, T], F32R)
            nc.sync.dma_start(out=ones_sb[:], in_=ones[:])
            bqk_sb = singles.tile([1, 2 * DL], F32R)
            nc.sync.dma_start(out=bqk_sb[:], in_=b_qk[:])
            bv_sb = singles.tile([1, DV], F32R)
            nc.sync.dma_start(out=bv_sb[:], in_=b_v[:])

            wqk_sb = []
            wv_sb = []
            for ci in range(8):
                t_qk = singles.tile([128, 2 * DL], F32R, name=f"wqk{ci}")
                nc.sync.dma_start(out=t_qk[:], in_=w_qk[ci * 128:(ci + 1) * 128, :])
                wqk_sb.append(t_qk)
                t_v = singles.tile([128, DV], F32R, name=f"wv{ci}")
                nc.sync.dma_start(out=t_v[:], in_=w_v[ci * 128:(ci + 1) * 128, :])
                wv_sb.append(t_v)
            wp_sb = []
            for di in range(2):
                t_p = singles.tile([128, C], F16, name=f"wp{di}")
                nc.sync.dma_start(out=t_p[:], in_=w_p[di * 128:(di + 1) * 128, :])
                wp_sb.append(t_p)

            # ---- Phase A: x -> X_T (transposed into [C, T]) -----------------
            xt = [singles.tile([128, T], F32R, name=f"xt{ci}") for ci in range(8)]
            ncopy = 0
            for tg in range(4):          # t groups of 512
                xa = []
                for k in range(4):       # t tiles of 128 within group
                    tt = tg * 4 + k
                    a = xload.tile([128, C], F32)
                    nc.sync.dma_start(out=a[:], in_=x_in[tt * 128:(tt + 1) * 128, :])
                    xa.append(a)
                for ci in range(8):
                    ps = psmm.tile([128, 512], F32, tag="mm")
                    for k in range(4):
                        nc.tensor.transpose(
                            ps[:, k * 128:(k + 1) * 128],
                            xa[k][:, ci * 128:(ci + 1) * 128],
                            iden_sb[:],
                        )
                    dst = xt[ci][:, tg * 512:(tg + 1) * 512]
                    if ncopy % 2 == 0:
                        nc.scalar.copy(dst, ps[:])
                    else:
                        nc.vector.tensor_copy(dst, ps[:])
                    ncopy += 1

            # ---- Phase B: QKV ----------------------------------------------
            # Q_T/K_T transposed [2*DL, T]: 4 tiles of [128, T]
            qkt = [singles.tile([128, T], F32R, name=f"qkt{m}") for m in range(4)]
            for m in range(4):
                for tb in range(4):
                    ps = psmm.tile([128, 512], F32, tag="mm")
                    for ci in range(8):
                        nc.tensor.matmul(
                            ps[:],
                            lhsT=wqk_sb[ci][:, m * 128:(m + 1) * 128],
                            rhs=xt[ci][:, tb * 512:(tb + 1) * 512],
                            start=(ci == 0),
                            stop=False,
                        )
                    nc.tensor.matmul(
                        ps[:],
                        lhsT=bqk_sb[:, m * 128:(m + 1) * 128],
                        rhs=ones_sb[:, :512],
                        start=False,
                        stop=True,
                    )
                    dst = qkt[m][:, tb * 512:(tb + 1) * 512]
                    if ncopy % 2 == 0:
                        nc.scalar.copy(dst, ps[:])
                    else:
                        nc.vector.tensor_copy(dst, ps[:])
                    ncopy += 1

            # V natural layout [T, DV] fp16: 16 tiles of [128, DV]
            v_sb = [singles.tile([128, DV], F16, name=f"v{tt}") for tt in range(16)]
            for tt in range(16):
                ps = psmm.tile([128, DV], F32, tag="mm")
                for ci in range(8):
                    nc.tensor.matmul(
                        ps[:],
                        lhsT=xt[ci][:, tt * 128:(tt + 1) * 128],
                        rhs=wv_sb[ci][:],
                        start=(ci == 0),
                        stop=False,
                    )
                nc.tensor.matmul(
                    ps[:],
                    lhsT=ones_sb[:, :128],
                    rhs=bv_sb[:],
                    start=False,
                    stop=True,
                )
                nc.vector.tensor_copy(v_sb[tt][:], ps[:])

            # ---- Phase C: attention ----------------------------------------
            # O_T fp16 [2*... DL, T]: 2 tiles of [128, T]
            ot = [singles.tile([128, T], F16, name=f"ot{di}") for di in range(2)]
            for h in range(HG):
                q_tile = qkt[h // 2]
                k_tile = qkt[2 + h // 2]
                moff = (h % 2) * 64
                for qb in range(4):      # q blocks of 512
                    avs = [psav.tile([128, DH + 1], F32, tag="av", name=f"av{h}_{qb}_{i}")
                           for i in range(4)]
                    pts = []
                    # software-pipelined: scores(st) ... av(st-1)
                    for st in range(17):
                        if st < 16:
                            ps = psmm.tile([128, 512], F32, tag="mm")
                            nc.tensor.matmul(
                                ps[:],
                                lhsT=k_tile[moff:moff + 64, st * 128:(st + 1) * 128],
                                rhs=q_tile[moff:moff + 64, qb * 512:(qb + 1) * 512],
                                start=True,
                                stop=True,
                            )
                            pt = ptp.tile([128, 512], F16)
                            nc.scalar.activation(pt[:], ps[:], Exp, scale=SCALE)
                            pts.append(pt)
                        if st > 0:
                            sp = st - 1
                            ptk = pts[sp]
                            for qs in range(4):
                                nc.tensor.matmul(
                                    avs[qs][:],
                                    lhsT=ptk[:, qs * 128:(qs + 1) * 128],
                                    rhs=v_sb[sp][:, h * (DH + 1):(h + 1) * (DH + 1)],
                                    start=(sp == 0),
                                    stop=(sp == 15),
                                )
                    for qs in range(4):
                        rec = osmall.tile([128, 1], F32, tag="rec")
                        nc.vector.reciprocal(rec[:], avs[qs][:, DH:DH + 1])
                        o_sb = osmall.tile([128, DH], F16, tag="o")
                        nc.vector.tensor_scalar_mul(o_sb[:], avs[qs][:, 0:DH], rec[:])
                        pst = psmm.tile([64, 128], F16, tag="mm")
                        nc.tensor.transpose(pst[:], o_sb[:], iden16[:])
                        dst = ot[h // 2][moff:moff + 64,
                                         qb * 512 + qs * 128:qb * 512 + (qs + 1) * 128]
                        if ncopy % 2 == 0:
                            nc.scalar.copy(dst, pst[:])
                        else:
                            nc.vector.tensor_copy(dst, pst[:])
                        ncopy += 1

            # ---- Phase D: output projection --------------------------------
            for tt in range(16):
                o_out = xload.tile([128, C], F32, tag="oout", name=f"oout{tt}")
                for nb in range(2):
                    ps = psmm.tile([128, 512], F32, tag="mm")
                    for di in range(2):
                        nc.tensor.matmul(
                            ps[:],
                            lhsT=ot[di][:, tt * 128:(tt + 1) * 128],
                            rhs=wp_sb[di][:, nb * 512:(nb + 1) * 512],
                            start=(di == 0),
                            stop=(di == 1),
                        )
                    dst = o_out[:, nb * 512:(nb + 1) * 512]
                    if ncopy % 2 == 0:
                        nc.scalar.copy(dst, ps[:])
                    else:
                        nc.vector.tensor_copy(dst, ps[:])
                    ncopy += 1
                nc.sync.dma_start(
                    out=out[tt * 128:(tt + 1) * 128, :],
                    in_=o_out[:],
                )

    nc.compile()
    return nc


_CACHE = {}


def _get_nc():
    if "nc" not in _CACHE:
        _CACHE["nc"] = build_bass()
    return _CACHE["nc"]


def make_in_maps(x, w_qkv, b_qkv, w_proj):
    identity = np.eye(128, dtype=np.float32)
    ones_row = np.ones((1, T), dtype=np.float32)
    in_maps = []
    for core in range(N_CORES):
        b = core // 4
        hg = core % 4
        cs = slice(hg * DL, (hg + 1) * DL)
        wq = w_qkv[:, 0 * C:1 * C][:, cs]
        wk = w_qkv[:, 1 * C:2 * C][:, cs]
        wv = w_qkv[:, 2 * C:3 * C][:, cs]
        bq = b_qkv[0 * C:1 * C][cs]
        bk = b_qkv[1 * C:2 * C][cs]
        bv = b_qkv[2 * C:3 * C][cs]
        # v extended: per head 64 v-cols + a ones column (softmax denominator)
        w_v_ext = np.zeros((C, DV), dtype=np.float32)
        b_v_ext = np.zeros((1, DV), dtype=np.float32)
        for hh in range(HG):
            w_v_ext[:, hh * (DH + 1):hh * (DH + 1) + DH] = wv[:, hh * DH:(hh + 1) * DH]
            b_v_ext[0, hh * (DH + 1):hh * (DH + 1) + DH] = bv[hh * DH:(hh + 1) * DH]
            b_v_ext[0, hh * (DH + 1) + DH] = 1.0
        in_maps.append({
            "x_loc": np.ascontiguousarray(x[b]),
            "w_qk": np.ascontiguousarray(np.concatenate([wq, wk], axis=1)),
            "w_v_ext": w_v_ext,
            "b_qk": np.concatenate([bq, bk])[None, :].astype(np.float32),
            "b_v_ext": b_v_ext,
            "w_proj_loc": np.ascontiguousarray(w_proj[cs, :]).astype(np.float16),
            "identity": identity,
            "ones_row": ones_row,
        })
    return in_maps


def kernel(x, w_qkv, b_qkv, w_proj, b_proj, **runner_kwargs):
    x = np.asarray(x, dtype=np.float32)
    w_qkv = np.asarray(w_qkv, dtype=np.float32)
    b_qkv = np.asarray(b_qkv, dtype=np.float32)
    w_proj = np.asarray(w_proj, dtype=np.float32)
    b_proj = np.asarray(b_proj, dtype=np.float32)

    nc = _get_nc()
    in_maps = make_in_maps(x, w_qkv, b_qkv, w_proj)
    res = run_bass_kernel_spmd(nc, in_maps, list(range(N_CORES)), **runner_kwargs)
    parts = [res.results[i]["out_partial"] for i in range(N_CORES)]
    outv = np.zeros((B, T, C), dtype=np.float32)
    for b in range(B):
        outv[b] = parts[4 * b + 0] + parts[4 * b + 1] + parts[4 * b + 2] + parts[4 * b + 3]
        outv[b] += b_proj[None, :]
    if runner_kwargs:
        return outv, res
    return outv


if __name__ == "__main__":
    import reference

    inputs = reference.setup_inputs()
    inputs = {k: np.asarray(v) for k, v in inputs.items()}
    got = kernel(**inputs)
    want = np.asarray(reference.reference(**inputs))
    err = np.abs(got - want).max() / np.abs(want).max()
    print("rel err:", err)


# revision 31
# speedup vs baseline: 8.7274x; 8.7274x over previous
"""Multi-head self-attention Trainium2 kernel, sharded over 8 NeuronCores.

Sharding: core = (batch, head_group): 2 batches x 4 head-groups (4 heads each).
Each core computes qkv for its batch restricted to its heads (tensor-parallel
column slice), full-sequence attention for those heads, and a row-parallel
slice of the output projection, producing a partial [T, C] output.
Host: out[b] = sum of the 4 head-group partials + b_proj.
"""

import math
import sys

import numpy as np

sys.path.insert(0, "/opt/trn_rl_repo")

import concourse.bacc as bacc
import concourse.bass as bass
import concourse.tile as tile
from concourse import mybir
from concourse.bass_utils import run_bass_kernel_spmd

B, T, C = 2, 2048, 1024
NH, DH = 16, 64
HG = 4                 # heads per core
DL = HG * DH           # 256 local qk channels
DV = HG * (DH + 1)     # 260: v columns + one ones-column per head
N_CORES = 8

F32 = mybir.dt.float32
F32R = mybir.dt.float32r
F16 = mybir.dt.float16

SCALE = 1.0 / math.sqrt(DH)


def build_bass():
    nc = bacc.Bacc("TRN2", target_bir_lowering=False, debug=False)

    x_in = nc.declare_dram_parameter("x_loc", [T, C], F32, isOutput=False)
    w_qk = nc.declare_dram_parameter("w_qk", [C, 2 * DL], F32R, isOutput=False)
    w_v = nc.declare_dram_parameter("w_v_ext", [C, DV], F32R, isOutput=False)
    b_qk = nc.declare_dram_parameter("b_qk", [128, 4], F32, isOutput=False)
    b_v = nc.declare_dram_parameter("b_v_ext", [1, DV], F32R, isOutput=False)
    w_p = nc.declare_dram_parameter("w_proj_loc", [DL, C], F16, isOutput=False)
    iden = nc.declare_dram_parameter("identity", [128, 128], F32, isOutput=False)
    ones = nc.declare_dram_parameter("ones_row", [1, T], F32R, isOutput=False)
    out = nc.declare_dram_parameter("out_partial", [T, C], F32, isOutput=True)

    Exp = mybir.ActivationFunctionType.Exp

    with tile.TileContext(nc) as tc:
        with (
            tc.tile_pool(name="singles", bufs=1) as singles,
            tc.tile_pool(name="xload", bufs=5) as xload,
            tc.tile_pool(name="pt", bufs=4) as ptp,
            tc.tile_pool(name="osmall", bufs=2) as osmall,
            tc.tile_pool(name="psmm", bufs=2, space="PSUM") as psmm,
            # av shares psmm
            tc.tile_pool(name="pssc", bufs=3, space="PSUM") as pssc,
        ):
            iden_sb = singles.tile([128, 128], F32)
            nc.sync.dma_start(out=iden_sb[:], in_=iden[:])
            ones_sb = singles.tile([1, T], F32R)
            nc.sync.dma_start(out=ones_sb[:], in_=ones[:])
            bqk_sb = singles.tile([128, 4], F32)
            nc.sync.dma_start(out=bqk_sb[:], in_=b_qk[:])
            bv_sb = singles.tile([1, DV], F32R)
            nc.sync.dma_start(out=bv_sb[:], in_=b_v[:])
            # broadcast V bias row to all 128 partitions (done once)
            bvb_ps = psmm.tile([128, DV], F32, tag="mm", name="bvb_ps")
            nc.tensor.matmul(
                bvb_ps[:], lhsT=ones_sb[:, :128], rhs=bv_sb[:],
                start=True, stop=True,
            )
            bvb = singles.tile([128, DV], F32)
            nc.vector.tensor_copy(bvb[:], bvb_ps[:])

            xa0 = []
            for k in range(4):
                a0 = xload.tile([128, C], F32, tag="xa", bufs=4, name=f"xa0_{k}")
                nc.sync.dma_start(out=a0[:], in_=x_in[k * 128:(k + 1) * 128, :])
                xa0.append(a0)

            wqk_sb = []
            wv_sb = []
            for ci in range(8):
                t_qk = singles.tile([128, 2 * DL], F32R, name=f"wqk{ci}")
                nc.sync.dma_start(out=t_qk[:], in_=w_qk[ci * 128:(ci + 1) * 128, :])
                wqk_sb.append(t_qk)
                t_v = singles.tile([128, DV], F32R, name=f"wv{ci}")
                nc.sync.dma_start(out=t_v[:], in_=w_v[ci * 128:(ci + 1) * 128, :])
                wv_sb.append(t_v)
            wp_sb = []
            for di in range(2):
                t_p = singles.tile([128, C], F16, name=f"wp{di}")
                nc.sync.dma_start(out=t_p[:], in_=w_p[di * 128:(di + 1) * 128, :])
                wp_sb.append(t_p)

            # ---- Phases A+B streamed per t-block of 512 ------------------
            xt = [singles.tile([128, T], F32R, name=f"xt{ci}") for ci in range(8)]
            qkt = [singles.tile([128, T], F32R, name=f"qkt{m}") for m in range(4)]
            v_sb = [singles.tile([128, DV], F16, name=f"v{tt}") for tt in range(16)]

            def qk_block(m, tb):
                ps = pssc.tile([128, 512], F32, tag="sc", name=f"qkps{m}_{tb}")
                for ci in range(8):
                    nc.tensor.matmul(
                        ps[:],
                        lhsT=wqk_sb[ci][:, m * 128:(m + 1) * 128],
                        rhs=xt[ci][:, tb * 512:(tb + 1) * 512],
                        start=(ci == 0),
                        stop=(ci == 7),
                    )
                dst = qkt[m][:, tb * 512:(tb + 1) * 512]
                nc.vector.tensor_scalar_add(dst, ps[:], bqk_sb[:, m:m + 1])

            def tb_group(tb):
                # load + transpose x for this t block
                if tb == 0:
                    xa = xa0
                else:
                    xa = []
                    for k in range(4):
                        tt = tb * 4 + k
                        a = xload.tile([128, C], F32, tag="xa", bufs=4,
                                       name=f"xa{tb}_{k}")
                        nc.sync.dma_start(out=a[:], in_=x_in[tt * 128:(tt + 1) * 128, :])
                        xa.append(a)
                for ci in range(8):
                    ps = pssc.tile([128, 512], F32, tag="sc", name=f"tp{tb}_{ci}")
                    for k in range(4):
                        nc.tensor.transpose(
                            ps[:, k * 128:(k + 1) * 128],
                            xa[k][:, ci * 128:(ci + 1) * 128],
                            iden_sb[:],
                        )
                    nc.vector.tensor_copy(
                        xt[ci][:, tb * 512:(tb + 1) * 512], ps[:])
                # K projections for this t block (attention consumes K in st order)
                qk_block(2, tb)
                qk_block(3, tb)
                # V for this t block
                for tt in range(tb * 4, tb * 4 + 4):
                    ps = pssc.tile([128, DV], F32, tag="sc", name=f"vps{tt}")
                    for ci in range(8):
                        nc.tensor.matmul(
                            ps[:],
                            lhsT=xt[ci][:, tt * 128:(tt + 1) * 128],
                            rhs=wv_sb[ci][:],
                            start=(ci == 0),
                            stop=(ci == 7),
                        )
                    nc.vector.tensor_add(v_sb[tt][:], ps[:], bvb[:])

            # ---- Phases C+D interleaved over q-blocks ----------------------
            # O_T fp16 [2*DL, T]: 2 tiles of [128, T]
            ot = [singles.tile([128, T], F16, name=f"ot{di}") for di in range(2)]
            def proj_tt(tt):
                o_out = xload.tile([128, C], F32, tag="oout", name=f"oout{tt}", bufs=3)
                for nb in range(2):
                    ps = psmm.tile([128, 512], F32, tag="mm", name=f"prps{tt}_{nb}")
                    for di in range(2):
                        nc.tensor.matmul(
                            ps[:],
                            lhsT=ot[di][:, tt * 128:(tt + 1) * 128],
                            rhs=wp_sb[di][:, nb * 512:(nb + 1) * 512],
                            start=(di == 0),
                            stop=(di == 1),
                        )
                    nc.vector.tensor_copy(o_out[:, nb * 512:(nb + 1) * 512], ps[:])
                nc.sync.dma_start(
                    out=out[tt * 128:(tt + 1) * 128, :],
                    in_=o_out[:],
                )

            pending_epi = [None]

            def emit_epilogue():
                if pending_epi[0] is None:
                    return
                h, qb, av = pending_epi[0]
                pending_epi[0] = None
                # divide by softmax sums (row 64), write O_T
                sums_sb = osmall.tile([1, 512], F32R, tag="sums")
                nc.vector.tensor_copy(sums_sb[:], av[DH:DH + 1, :])
                bc = pssc.tile([64, 512], F32, tag="sc", name=f"bc{h}_{qb}")
                nc.tensor.matmul(
                    bc[:],
                    lhsT=ones_sb[:, :64],
                    rhs=sums_sb[:],
                    start=True,
                    stop=True,
                )
                rec = osmall.tile([64, 512], F32, tag="rec")
                nc.vector.reciprocal(rec[:], bc[:])
                nc.vector.tensor_mul(
                    ot[h // 2][moff_of(h):moff_of(h) + 64,
                               qb * 512:(qb + 1) * 512],
                    av[0:DH, :],
                    rec[:],
                )

            def moff_of(h):
                return (h % 2) * 64

            LOOK = 2   # pairs of lookahead between scores/exp and AV

            class Unit:
                def __init__(self, h, qb):
                    self.h, self.qb = h, qb
                    self.q_tile = qkt[h // 2]
                    self.k_tile = qkt[2 + h // 2]
                    self.moff = moff_of(h)
                    self.av = None
                    self.pts = []
                    self.sc_done = 0
                    self.av_done = 0

                def _emit_scores_pair(self):
                    p = self.sc_done
                    h, qb = self.h, self.qb
                    ps = pssc.tile([128, 1024], F32, tag="sc",
                                   name=f"sc{h}_{qb}_{p}")
                    for half in range(2):
                        st = 2 * p + half
                        nc.tensor.matmul(
                            ps[:, half * 512:(half + 1) * 512],
                            lhsT=self.k_tile[self.moff:self.moff + 64,
                                             st * 128:(st + 1) * 128],
                            rhs=self.q_tile[self.moff:self.moff + 64,
                                            qb * 512:(qb + 1) * 512],
                            start=True,
                            stop=True,
                        )
                    pt = ptp.tile([128, 1024], F16, tag="pt", name=f"pt{h}_{qb}_{p}")
                    nc.scalar.activation(pt[:], ps[:], Exp, scale=SCALE)
                    self.pts.append(pt)
                    self.sc_done += 1

                def _emit_av_pair(self):
                    sp = self.av_done
                    h, qb = self.h, self.qb
                    if self.av is None:
                        self.av = psmm.tile([DH + 1, 512], F32, tag="mm",
                                            name=f"av{h}_{qb}")
                    ptk = self.pts[sp]
                    for half in range(2):
                        st = 2 * sp + half
                        nc.tensor.matmul(
                            self.av[:],
                            lhsT=v_sb[st][:, h * (DH + 1):(h + 1) * (DH + 1)],
                            rhs=ptk[:, half * 512:(half + 1) * 512],
                            start=(st == 0),
                            stop=(st == 15),
                        )
                    self.av_done += 1

                def emit(self, n_pairs):
                    for _ in range(n_pairs):
                        if self.sc_done < 8:
                            self._emit_scores_pair()
                        if self.sc_done == 2 and self.av_done == 0:
                            emit_epilogue()
                        if self.sc_done - self.av_done > LOOK or \
                           (self.sc_done == 8 and self.av_done < 8 and
                            self.sc_done - self.av_done > LOOK):
                            self._emit_av_pair()

                def finish(self):
                    while self.sc_done < 8 or self.av_done < 8:
                        if self.sc_done < 8:
                            self._emit_scores_pair()
                            if self.sc_done == 2 and self.av_done == 0:
                                emit_epilogue()
                        else:
                            self._emit_av_pair()
                    emit_epilogue()
                    pending_epi[0] = (self.h, self.qb, self.av)

            def attn_unit(h, qb):
                u = Unit(h, qb)
                u.finish()

            tb_group(0)
            qk_block(0, 0)
            qk_block(1, 0)
            u00 = Unit(0, 0)
            u10 = Unit(1, 0)
            u00.emit(2)
            tb_group(1)
            u00.emit(2)
            u10.emit(2)
            tb_group(2)
            u00.emit(2)
            u10.emit(2)
            tb_group(3)
            u00.emit(2)
            u10.emit(2)
            u00.finish()
            u10.finish()
            for qb in range(4):          # q blocks of 512
                for h in range(HG):
                    if not (qb == 0 and h < 2):
                        attn_unit(h, qb)
                    # PE filler while ACT runs exp:
                    if h == 0 and qb < 3:
                        qk_block(0, qb + 1)
                    if h == 1 and qb < 3:
                        qk_block(1, qb + 1)
                    if h >= 2 and qb > 0:
                        base = (qb - 1) * 4 + (h - 2) * 2
                        proj_tt(base)
                        proj_tt(base + 1)
            emit_epilogue()
            for tt in range(12, 16):
                proj_tt(tt)

    nc.compile()
    return nc


_CACHE = {}


def _get_nc():
    if "nc" not in _CACHE:
        _CACHE["nc"] = build_bass()
    return _CACHE["nc"]


def make_in_maps(x, w_qkv, b_qkv, w_proj):
    identity = np.eye(128, dtype=np.float32)
    ones_row = np.ones((1, T), dtype=np.float32)
    in_maps = []
    for core in range(N_CORES):
        b = core // 4
        hg = core % 4
        cs = slice(hg * DL, (hg + 1) * DL)
        wq = w_qkv[:, 0 * C:1 * C][:, cs]
        wk = w_qkv[:, 1 * C:2 * C][:, cs]
        wv = w_qkv[:, 2 * C:3 * C][:, cs]
        bq = b_qkv[0 * C:1 * C][cs]
        bk = b_qkv[1 * C:2 * C][cs]
        bv = b_qkv[2 * C:3 * C][cs]
        # v extended: per head 64 v-cols + a ones column (softmax denominator)
        w_v_ext = np.zeros((C, DV), dtype=np.float32)
        b_v_ext = np.zeros((1, DV), dtype=np.float32)
        for hh in range(HG):
            w_v_ext[:, hh * (DH + 1):hh * (DH + 1) + DH] = wv[:, hh * DH:(hh + 1) * DH]
            b_v_ext[0, hh * (DH + 1):hh * (DH + 1) + DH] = bv[hh * DH:(hh + 1) * DH]
            b_v_ext[0, hh * (DH + 1) + DH] = 1.0
        in_maps.append({
            "x_loc": np.ascontiguousarray(x[b]),
            "w_qk": np.ascontiguousarray(np.concatenate([wq, wk], axis=1)),
            "w_v_ext": w_v_ext,
            "b_qk": np.stack(
                [np.concatenate([bq, bk])[m * 128:(m + 1) * 128] for m in range(4)],
                axis=1).astype(np.float32),
            "b_v_ext": b_v_ext,
            "w_proj_loc": np.ascontiguousarray(w_proj[cs, :]).astype(np.float16),
            "identity": identity,
            "ones_row": ones_row,
        })
    return in_maps


def kernel(x, w_qkv, b_qkv, w_proj, b_proj, **runner_kwargs):
    x = np.asarray(x, dtype=np.float32)
    w_qkv = np.asarray(w_qkv, dtype=np.float32)
    b_qkv = np.asarray(b_qkv, dtype=np.float32)
    w_proj = np.asarray(w_proj, dtype=np.float32)
    b_proj = np.asarray(b_proj, dtype=np.float32)

    nc = _get_nc()
    in_maps = make_in_maps(x, w_qkv, b_qkv, w_proj)
    res = run_bass_kernel_spmd(nc, in_maps, list(range(N_CORES)), **runner_kwargs)
    parts = [res.results[i]["out_partial"] for i in range(N_CORES)]
    outv = np.zeros((B, T, C), dtype=np.float32)
    for b in range(B):
        outv[b] = parts[4 * b + 0] + parts[4 * b + 1] + parts[4 * b + 2] + parts[4 * b + 3]
        outv[b] += b_proj[None, :]
    if runner_kwargs:
        return outv, res
    return outv


if __name__ == "__main__":
    import reference

    inputs = reference.setup_inputs()
    inputs = {k: np.asarray(v) for k, v in inputs.items()}
    got = kernel(**inputs)
    want = np.asarray(reference.reference(**inputs))
    err = np.abs(got - want).max() / np.abs(want).max()
    print("rel err:", err)


# revision 32
# speedup vs baseline: 8.7386x; 1.0013x over previous
"""Multi-head self-attention Trainium2 kernel, sharded over 8 NeuronCores.

Sharding: core = (batch, head_group): 2 batches x 4 head-groups (4 heads each).
Each core computes qkv for its batch restricted to its heads (tensor-parallel
column slice), full-sequence attention for those heads, and a row-parallel
slice of the output projection, producing a partial [T, C] output.
Host: out[b] = sum of the 4 head-group partials + b_proj.
"""

import math
import sys

import numpy as np

sys.path.insert(0, "/opt/trn_rl_repo")

import concourse.bacc as bacc
import concourse.bass as bass
import concourse.tile as tile
from concourse import mybir
from concourse.bass_utils import run_bass_kernel_spmd

B, T, C = 2, 2048, 1024
NH, DH = 16, 64
HG = 4                 # heads per core
DL = HG * DH           # 256 local qk channels
DV = HG * (DH + 1)     # 260: v columns + one ones-column per head
N_CORES = 8

F32 = mybir.dt.float32
F32R = mybir.dt.float32r
F16 = mybir.dt.float16

SCALE = 1.0 / math.sqrt(DH)


def build_bass():
    nc = bacc.Bacc("TRN2", target_bir_lowering=False, debug=False)

    x_in = nc.declare_dram_parameter("x_loc", [T, C], F32, isOutput=False)
    w_qk = nc.declare_dram_parameter("w_qk", [C, 2 * DL], F32R, isOutput=False)
    w_v = nc.declare_dram_parameter("w_v_ext", [C, DV], F32R, isOutput=False)
    b_qk = nc.declare_dram_parameter("b_qk", [128, 4], F32, isOutput=False)
    b_v = nc.declare_dram_parameter("b_v_ext", [1, DV], F32R, isOutput=False)
    w_p = nc.declare_dram_parameter("w_proj_loc", [DL, C], F16, isOutput=False)
    iden = nc.declare_dram_parameter("identity", [128, 128], F32, isOutput=False)
    ones = nc.declare_dram_parameter("ones_row", [1, T], F32R, isOutput=False)
    out = nc.declare_dram_parameter("out_partial", [T, C], F32, isOutput=True)

    Exp = mybir.ActivationFunctionType.Exp

    with tile.TileContext(nc) as tc:
        with (
            tc.tile_pool(name="singles", bufs=1) as singles,
            tc.tile_pool(name="xload", bufs=5) as xload,
            tc.tile_pool(name="pt", bufs=5) as ptp,
            tc.tile_pool(name="osmall", bufs=3) as osmall,
            tc.tile_pool(name="psmm", bufs=2, space="PSUM") as psmm,
            # av shares psmm
            tc.tile_pool(name="pssc", bufs=3, space="PSUM") as pssc,
        ):
            iden_sb = singles.tile([128, 128], F32)
            nc.sync.dma_start(out=iden_sb[:], in_=iden[:])
            ones_sb = singles.tile([1, T], F32R)
            nc.sync.dma_start(out=ones_sb[:], in_=ones[:])
            bqk_sb = singles.tile([128, 4], F32)
            nc.sync.dma_start(out=bqk_sb[:], in_=b_qk[:])
            bv_sb = singles.tile([1, DV], F32R)
            nc.sync.dma_start(out=bv_sb[:], in_=b_v[:])
            # broadcast V bias row to all 128 partitions (done once)
            bvb_ps = psmm.tile([128, DV], F32, tag="mm", name="bvb_ps")
            nc.tensor.matmul(
                bvb_ps[:], lhsT=ones_sb[:, :128], rhs=bv_sb[:],
                start=True, stop=True,
            )
            bvb = singles.tile([128, DV], F32)
            nc.vector.tensor_copy(bvb[:], bvb_ps[:])

            xa0 = []
            for k in range(4):
                a0 = xload.tile([128, C], F32, tag="xa", bufs=4, name=f"xa0_{k}")
                nc.sync.dma_start(out=a0[:], in_=x_in[k * 128:(k + 1) * 128, :])
                xa0.append(a0)

            wqk_sb = []
            wv_sb = []
            for ci in range(8):
                t_qk = singles.tile([128, 2 * DL], F32R, name=f"wqk{ci}")
                nc.sync.dma_start(out=t_qk[:], in_=w_qk[ci * 128:(ci + 1) * 128, :])
                wqk_sb.append(t_qk)
                t_v = singles.tile([128, DV], F32R, name=f"wv{ci}")
                nc.sync.dma_start(out=t_v[:], in_=w_v[ci * 128:(ci + 1) * 128, :])
                wv_sb.append(t_v)
            wp_sb = []
            for di in range(2):
                t_p = singles.tile([128, C], F16, name=f"wp{di}")
                nc.sync.dma_start(out=t_p[:], in_=w_p[di * 128:(di + 1) * 128, :])
                wp_sb.append(t_p)

            # ---- Phases A+B streamed per t-block of 512 ------------------
            xt = [singles.tile([128, T], F32R, name=f"xt{ci}") for ci in range(8)]
            qkt = [singles.tile([128, T], F32R, name=f"qkt{m}") for m in range(4)]
            v_sb = [singles.tile([128, DV], F16, name=f"v{tt}") for tt in range(16)]

            def qk_block(m, tb):
                ps = pssc.tile([128, 512], F32, tag="sc", name=f"qkps{m}_{tb}")
                for ci in range(8):
                    nc.tensor.matmul(
                        ps[:],
                        lhsT=wqk_sb[ci][:, m * 128:(m + 1) * 128],
                        rhs=xt[ci][:, tb * 512:(tb + 1) * 512],
                        start=(ci == 0),
                        stop=(ci == 7),
                    )
                dst = qkt[m][:, tb * 512:(tb + 1) * 512]
                nc.vector.tensor_scalar_add(dst, ps[:], bqk_sb[:, m:m + 1])

            def tb_group(tb):
                # load + transpose x for this t block
                if tb == 0:
                    xa = xa0
                else:
                    xa = []
                    for k in range(4):
                        tt = tb * 4 + k
                        a = xload.tile([128, C], F32, tag="xa", bufs=4,
                                       name=f"xa{tb}_{k}")
                        nc.sync.dma_start(out=a[:], in_=x_in[tt * 128:(tt + 1) * 128, :])
                        xa.append(a)
                for ci in range(8):
                    ps = pssc.tile([128, 512], F32, tag="sc", name=f"tp{tb}_{ci}")
                    for k in range(4):
                        nc.tensor.transpose(
                            ps[:, k * 128:(k + 1) * 128],
                            xa[k][:, ci * 128:(ci + 1) * 128],
                            iden_sb[:],
                        )
                    nc.vector.tensor_copy(
                        xt[ci][:, tb * 512:(tb + 1) * 512], ps[:])
                # K projections for this t block (attention consumes K in st order)
                qk_block(2, tb)
                qk_block(3, tb)
                # V for this t block
                for tt in range(tb * 4, tb * 4 + 4):
                    ps = pssc.tile([128, DV], F32, tag="sc", name=f"vps{tt}")
                    for ci in range(8):
                        nc.tensor.matmul(
                            ps[:],
                            lhsT=xt[ci][:, tt * 128:(tt + 1) * 128],
                            rhs=wv_sb[ci][:],
                            start=(ci == 0),
                            stop=(ci == 7),
                        )
                    nc.vector.tensor_add(v_sb[tt][:], ps[:], bvb[:])

            # ---- Phases C+D interleaved over q-blocks ----------------------
            # O_T fp16 [2*DL, T]: 2 tiles of [128, T]
            ot = [singles.tile([128, T], F16, name=f"ot{di}") for di in range(2)]
            def proj_tt(tt):
                o_out = xload.tile([128, C], F32, tag="oout", name=f"oout{tt}", bufs=3)
                for nb in range(2):
                    ps = psmm.tile([128, 512], F32, tag="mm", name=f"prps{tt}_{nb}")
                    for di in range(2):
                        nc.tensor.matmul(
                            ps[:],
                            lhsT=ot[di][:, tt * 128:(tt + 1) * 128],
                            rhs=wp_sb[di][:, nb * 512:(nb + 1) * 512],
                            start=(di == 0),
                            stop=(di == 1),
                        )
                    nc.vector.tensor_copy(o_out[:, nb * 512:(nb + 1) * 512], ps[:])
                nc.sync.dma_start(
                    out=out[tt * 128:(tt + 1) * 128, :],
                    in_=o_out[:],
                )

            pending_epi = [None]

            def emit_epilogue():
                if pending_epi[0] is None:
                    return
                h, qb, av = pending_epi[0]
                pending_epi[0] = None
                # divide by softmax sums (row 64), write O_T
                sums_sb = osmall.tile([1, 512], F32R, tag="sums")
                nc.vector.tensor_copy(sums_sb[:], av[DH:DH + 1, :])
                bc = pssc.tile([64, 512], F32, tag="sc", name=f"bc{h}_{qb}")
                nc.tensor.matmul(
                    bc[:],
                    lhsT=ones_sb[:, :64],
                    rhs=sums_sb[:],
                    start=True,
                    stop=True,
                )
                rec = osmall.tile([64, 512], F32, tag="rec")
                nc.vector.reciprocal(rec[:], bc[:])
                nc.vector.tensor_mul(
                    ot[h // 2][moff_of(h):moff_of(h) + 64,
                               qb * 512:(qb + 1) * 512],
                    av[0:DH, :],
                    rec[:],
                )

            def moff_of(h):
                return (h % 2) * 64

            LOOK = 2   # pairs of lookahead between scores/exp and AV

            class Unit:
                def __init__(self, h, qb):
                    self.h, self.qb = h, qb
                    self.q_tile = qkt[h // 2]
                    self.k_tile = qkt[2 + h // 2]
                    self.moff = moff_of(h)
                    self.av = None
                    self.pts = []
                    self.sc_done = 0
                    self.av_done = 0

                def _emit_scores_pair(self):
                    p = self.sc_done
                    h, qb = self.h, self.qb
                    ps = pssc.tile([128, 1024], F32, tag="sc",
                                   name=f"sc{h}_{qb}_{p}")
                    for half in range(2):
                        st = 2 * p + half
                        nc.tensor.matmul(
                            ps[:, half * 512:(half + 1) * 512],
                            lhsT=self.k_tile[self.moff:self.moff + 64,
                                             st * 128:(st + 1) * 128],
                            rhs=self.q_tile[self.moff:self.moff + 64,
                                            qb * 512:(qb + 1) * 512],
                            start=True,
                            stop=True,
                        )
                    pt = ptp.tile([128, 1024], F16, tag="pt", name=f"pt{h}_{qb}_{p}")
                    nc.scalar.activation(pt[:], ps[:], Exp, scale=SCALE)
                    self.pts.append(pt)
                    self.sc_done += 1

                def _emit_av_pair(self):
                    sp = self.av_done
                    h, qb = self.h, self.qb
                    if self.av is None:
                        self.av = psmm.tile([DH + 1, 512], F32, tag="mm",
                                            name=f"av{h}_{qb}")
                    ptk = self.pts[sp]
                    for half in range(2):
                        st = 2 * sp + half
                        nc.tensor.matmul(
                            self.av[:],
                            lhsT=v_sb[st][:, h * (DH + 1):(h + 1) * (DH + 1)],
                            rhs=ptk[:, half * 512:(half + 1) * 512],
                            start=(st == 0),
                            stop=(st == 15),
                        )
                    self.av_done += 1

                def emit(self, n_pairs):
                    for _ in range(n_pairs):
                        if self.sc_done < 8:
                            self._emit_scores_pair()
                        if self.sc_done == 2 and self.av_done == 0:
                            emit_epilogue()
                        if self.sc_done - self.av_done > LOOK or \
                           (self.sc_done == 8 and self.av_done < 8 and
                            self.sc_done - self.av_done > LOOK):
                            self._emit_av_pair()

                def finish(self):
                    while self.sc_done < 8 or self.av_done < 8:
                        if self.sc_done < 8:
                            self._emit_scores_pair()
                            if self.sc_done == 2 and self.av_done == 0:
                                emit_epilogue()
                        else:
                            self._emit_av_pair()
                    emit_epilogue()
                    pending_epi[0] = (self.h, self.qb, self.av)

            def attn_unit(h, qb):
                u = Unit(h, qb)
                u.finish()

            tb_group(0)
            qk_block(0, 0)
            qk_block(1, 0)
            u00 = Unit(0, 0)
            u10 = Unit(1, 0)
            u00.emit(2)
            tb_group(1)
            u00.emit(2)
            u10.emit(2)
            tb_group(2)
            u00.emit(2)
            u10.emit(2)
            tb_group(3)
            u00.emit(2)
            u10.emit(2)
            u00.finish()
            u10.finish()
            for qb in range(4):          # q blocks of 512
                for h in range(HG):
                    if not (qb == 0 and h < 2):
                        attn_unit(h, qb)
                    # PE filler while ACT runs exp:
                    if h == 0 and qb < 3:
                        qk_block(0, qb + 1)
                    if h == 1 and qb < 3:
                        qk_block(1, qb + 1)
                    if h >= 2 and qb > 0:
                        base = (qb - 1) * 4 + (h - 2) * 2
                        proj_tt(base)
                        proj_tt(base + 1)
            emit_epilogue()
            for tt in range(12, 16):
                proj_tt(tt)

    nc.compile()
    return nc


_CACHE = {}


def _get_nc():
    if "nc" not in _CACHE:
        _CACHE["nc"] = build_bass()
    return _CACHE["nc"]


def make_in_maps(x, w_qkv, b_qkv, w_proj):
    identity = np.eye(128, dtype=np.float32)
    ones_row = np.ones((1, T), dtype=np.float32)
    in_maps = []
    for core in range(N_CORES):
        b = core // 4
        hg = core % 4
        cs = slice(hg * DL, (hg + 1) * DL)
        wq = w_qkv[:, 0 * C:1 * C][:, cs]
        wk = w_qkv[:, 1 * C:2 * C][:, cs]
        wv = w_qkv[:, 2 * C:3 * C][:, cs]
        bq = b_qkv[0 * C:1 * C][cs]
        bk = b_qkv[1 * C:2 * C][cs]
        bv = b_qkv[2 * C:3 * C][cs]
        # v extended: per head 64 v-cols + a ones column (softmax denominator)
        w_v_ext = np.zeros((C, DV), dtype=np.float32)
        b_v_ext = np.zeros((1, DV), dtype=np.float32)
        for hh in range(HG):
            w_v_ext[:, hh * (DH + 1):hh * (DH + 1) + DH] = wv[:, hh * DH:(hh + 1) * DH]
            b_v_ext[0, hh * (DH + 1):hh * (DH + 1) + DH] = bv[hh * DH:(hh + 1) * DH]
            b_v_ext[0, hh * (DH + 1) + DH] = 1.0
        in_maps.append({
            "x_loc": np.ascontiguousarray(x[b]),
            "w_qk": np.ascontiguousarray(np.concatenate([wq, wk], axis=1)),
            "w_v_ext": w_v_ext,
            "b_qk": np.stack(
                [np.concatenate([bq, bk])[m * 128:(m + 1) * 128] for m in range(4)],
                axis=1).astype(np.float32),
            "b_v_ext": b_v_ext,
            "w_proj_loc": np.ascontiguousarray(w_proj[cs, :]).astype(np.float16),
            "identity": identity,
            "ones_row": ones_row,
        })
    return in_maps


def kernel(x, w_qkv, b_qkv, w_proj, b_proj, **runner_kwargs):
    x = np.asarray(x, dtype=np.float32)
    w_qkv = np.asarray(w_qkv, dtype=np.float32)
    b_qkv = np.asarray(b_qkv, dtype=np.float32)
    w_proj = np.asarray(w_proj, dtype=np.float32)
    b_proj = np.asarray(b_proj, dtype=np.float32)

    nc = _get_nc()
    in_maps = make_in_maps(x, w_qkv, b_qkv, w_proj)
    res = run_bass_kernel_spmd(nc, in_maps, list(range(N_CORES)), **runner_kwargs)
    parts = [res.results[i]["out_partial"] for i in range(N_CORES)]
    outv = np.zeros((B, T, C), dtype=np.float32)
    for b in range(B):
        outv[b] = parts[4 * b + 0] + parts[4 * b + 1] + parts[4 * b + 2] + parts[4 * b + 3]
        outv[b] += b_proj[None, :]
    if runner_kwargs:
        return outv, res
    return outv


if __name__ == "__main__":
    import reference

    inputs = reference.setup_inputs()
    inputs = {k: np.asarray(v) for k, v in inputs.items()}
    got = kernel(**inputs)
    want = np.asarray(reference.reference(**inputs))
    err = np.abs(got - want).max() / np.abs(want).max()
    print("rel err:", err)
